# revision 14
# baseline (speedup 1.0000x reference)
"""Trainium2 Bass kernel for CenterWoParamMultiCosineSoftmaxLoss.

loss = mean_b sum_k softmax_k(2 - dst_bk) * dst_bk,
  dst_bk = 1 - <x_b/||x_b||, c_{l_b,k}/||c_{l_b,k}||>

Identities used:
  softmax(2 - dst) = softmax(s)     (shift invariance; s = cosine score)
  per_sample       = 1 - sum_k p_k s_k
  xT is pre-scaled by rnorm_b during the PSUM->SBUF pack copy, so the
  matmul emits normalized scores directly; Z and num come from one exp
  pass + segmented reduces.

Distribution: samples grouped by label on the host into 256-slot segments
(one class per segment), 12 segments per core, identical SPMD program on 8
cores. Pad slots are zero rows and contribute exactly 0.

Pipeline per core (slots=3072, 24 sub-chunks of 128 rows):
  - x DMAs land 2 segments at a time as [128, 4KB] lines (row pair 2p,2p+1
    per partition) - 4KB contiguous DRAM reads per partition line.
  - per sub-chunk: sum-of-squares (ACT Square+accum / DVE fused
    tensor_tensor_reduce, alternating), pair-batched rsqrt via ln/exp,
    4 fp32 PE transposes, scaled pack-copy (x * rnorm -> bf16 xT,
    ACT/DVE alternating), 4 bf16 accumulating score matmuls.
  - per group of 6 sub-chunks: one exp over [128,192], segmented Z and
    num reduces on DVE.
  - tail: t = num/Z batched, row reduce, PE ones-matmul, DMA out.
"""

import sys

for _p in ("/opt/trn_rl_repo", "/root/.axon_site/_ro/trn_rl_repo"):
    if _p not in sys.path:
        sys.path.append(_p)

import numpy as np

import concourse.bass as bass
import concourse.mybir as mybir
from concourse.tile import TileContext
from concourse.masks import make_identity
from concourse.bass_utils import run_bass_kernel_spmd
from concourse.vector_clock import ScopedClock

B, D, C, K = 16384, 512, 90, 32
NCORES = 8
SEGW = 256          # slots per segment (one class per segment), 2 chunks of 128
P = 128
DCH = D // P        # 4 contraction chunks
f32 = mybir.dt.float32
bf16 = mybir.dt.bfloat16
AF = mybir.ActivationFunctionType
ALU = mybir.AluOpType

_tile_patched = False


def _install_tile_patch():
    """This walrus build allows only one sem wait on TPB_CTRL-lowered
    instructions (Drain / sync-NoOp). Tile's tail drain attaches one wait per
    live processor clock; split them into a chain of single-wait NoOps."""
    global _tile_patched
    if _tile_patched:
        return
    _tile_patched = True

    def _drain_and_barrier(self, tick_clock, wait_clock):
        nc = self.nc
        probe = nc.sync.nop(nofuse=True)
        wait_clock.add_sem_waits(
            probe.ins, ScopedClock({None: tick_clock.global_clock})
        )
        si = probe.ins.sync_info
        if si is not None and len(si.on_wait) > 1:
            waits = list(si.on_wait)
            si.on_wait.clear()
            si.on_wait.append(waits[0])
            for w in waits[1:]:
                n2 = nc.sync.nop(nofuse=True)
                if n2.ins.sync_info is None:
                    n2.ins.sync_info = mybir.SyncInfo(on_wait=[w], on_update=[])
                else:
                    n2.ins.sync_info.on_wait.append(w)
        nc.sync.drain()
        nc.all_engine_barrier()
        assert self.sems is not None
        popped = nc._tile_sem_poison_stack.pop()
        assert popped is self._sem_poison
        nc.clear_and_free_semaphores(list(self.sems.allocated().values()))
        nc.all_engine_barrier()

    TileContext._drain_and_barrier = _drain_and_barrier


def _split_excess_waits(nc, max_waits=1):
    """This walrus build accepts at most one sem wait per instruction for
    several opcodes. Hoist excess waits onto single-wait NoOps emitted just
    before the instruction on the same engine (engine streams are serial, so
    semantics are preserved)."""
    n = 0
    for fn in nc.m.functions:
        for blk in fn.blocks:
            newl = []
            for inst in blk.instructions:
                si = getattr(inst, "sync_info", None)
                if si is not None and si.on_wait is not None and len(si.on_wait) > max_waits:
                    waits = list(si.on_wait)
                    keep = waits[-max_waits:]
                    extra = waits[:-max_waits]
                    si.on_wait.clear()
                    for w in keep:
                        si.on_wait.append(w)
                    for w in extra:
                        n += 1
                        newl.append(
                            mybir.InstNoOp(
                                name=f"{inst.name}-w{n}",
                                engine=inst.engine,
                                sync_info=mybir.SyncInfo(on_wait=[w], on_update=[]),
                                bass_nofuse=True,
                            )
                        )
                newl.append(inst)
            blk.instructions[:] = newl
    return nc


def build_bass(nseg: int, split_waits: bool = True):
    """One core's program: nseg segments of SEGW class-grouped sample slots."""
    _install_tile_patch()
    slots = nseg * SEGW
    nch = slots // P                  # sub-chunks of 128 rows (2 per segment)
    ck = nseg * K                     # center rows used
    ct = (ck + P - 1) // P            # center row tiles
    ckp = ct * P                      # padded center rows
    npair = nseg // 2                 # 2-segment DMA quanta
    odd = nseg % 2

    # softmax group = 6 sub-chunks -> one PSUM bank [128, 192] f32
    GRP = max(6, (nch + 3) // 4)
    ngrp = (nch + GRP - 1) // GRP

    nc = bass.Bass()
    xg = nc.dram_tensor("xg", [slots, D], f32, kind="ExternalInput")
    cent = nc.dram_tensor("cent", [ckp, D], f32, kind="ExternalInput")
    out = nc.dram_tensor("partial", [1, 1], f32, kind="ExternalOutput")

    with TileContext(nc) as tc:
        with (
            tc.tile_pool(name="const", bufs=1) as const_pool,
            tc.tile_pool(name="persist", bufs=1) as persist,
            tc.tile_pool(name="cin", bufs=1) as cin_pool,
            tc.tile_pool(name="cnb", bufs=4) as cnb_pool,
            tc.tile_pool(name="junk", bufs=6) as junk_pool,
            tc.tile_pool(name="esb", bufs=2) as esb_pool,
            tc.tile_pool(name="tp_ps", bufs=2, space="PSUM") as tp_psum,
            tc.tile_pool(name="sc_ps", bufs=1, space="PSUM") as sc_psum,
            tc.tile_pool(name="fin_ps", bufs=1, space="PSUM") as fin_psum,
        ):
            id_f32 = const_pool.tile([P, P], f32)
            make_identity(nc, id_f32[:])
            id_bf16 = const_pool.tile([P, P], bf16)
            make_identity(nc, id_bf16[:])
            ones = const_pool.tile([P, 1], f32)
            nc.gpsimd.memset(ones[:], 1.0)

            # persistent tensors
            xf = persist.tile([P, nch * D], f32)       # sub-chunk i at cols [i*D, +D)
            xT = persist.tile([P, DCH * slots], bf16)  # d-chunk c at cols [c*slots, +slots)
            cnT = persist.tile([P, DCH * ckp], bf16)   # d-chunk c at cols [c*ckp, +ckp)
            ssq = persist.tile([P, nch], f32)          # sum_d x^2 per sub-chunk col
            rnorm = persist.tile([P, nch], f32)        # rsqrt(ss + eps)
            zsum = persist.tile([P, nch], f32)         # softmax denominators
            nums = persist.tile([P, nch], f32)         # sum_k e_k * s_k
            c_ssr = persist.tile([P, ct], f32)         # ss_c then +eps
            c_ln = persist.tile([P, ct], f32)
            c_rn = persist.tile([P, ct], f32)          # rsqrt(ss_c + eps)

            # ---- all DMAs issued up front on the sync queue: the ring
            # streams centers then x pairs back to back.
            cfs = []
            for t in range(ct):
                cf = cin_pool.tile([P, D], f32, tag=f"cin{t}")
                cfs.append(cf)
                nc.sync.dma_start(out=cf[:], in_=cent[t * P:(t + 1) * P, :])
            for q in range(npair):
                src = xg[2 * q * SEGW:(2 * q + 2) * SEGW, :].rearrange(
                    "(s p r) d -> p s r d", s=2, p=P, r=2
                )
                dst = xf[:, q * 4 * D:(q + 1) * 4 * D].rearrange(
                    "p (s r d) -> p s r d", s=2, r=2, d=D
                )
                nc.sync.dma_start(out=dst, in_=src)
            if odd:
                src = xg[(nseg - 1) * SEGW:nseg * SEGW, :].rearrange(
                    "(p r) d -> p r d", p=P, r=2
                )
                dst = xf[:, npair * 4 * D:(npair * 4 + 2) * D].rearrange(
                    "p (r d) -> p r d", r=2, d=D
                )
                nc.sync.dma_start(out=dst, in_=src)

            # ---- centers: row sum-of-squares, rsqrt, scaled bf16 copy,
            # transpose into cnT.
            for t in range(ct):
                cjunk = junk_pool.tile([P, D], f32, tag="junkA")
                nc.scalar.activation(
                    out=cjunk[:], in_=cfs[t][:], func=AF.Square,
                    accum_out=c_ssr[:, t:t + 1],
                )
            nc.vector.tensor_scalar_add(out=c_ssr[:], in0=c_ssr[:], scalar1=1e-12)
            nc.scalar.activation(out=c_ln[:], in_=c_ssr[:], func=AF.Ln)
            nc.scalar.activation(out=c_rn[:], in_=c_ln[:], func=AF.Exp, scale=-0.5)
            for t in range(ct):
                cb = cnb_pool.tile([P, D], bf16, tag="cnb")
                nc.scalar.activation(
                    out=cb[:], in_=cfs[t][:], func=AF.Copy, scale=c_rn[:, t:t + 1],
                )
                cps = tp_psum.tile([P, D], bf16, tag="ctp", bufs=1)
                for c in range(DCH):
                    nc.tensor.transpose(
                        cps[:, c * P:(c + 1) * P], cb[:, c * P:(c + 1) * P],
                        id_bf16[:],
                    )
                nc.vector.tensor_copy(
                    out=cnT[:].rearrange("p (c n) -> p c n", c=DCH)[
                        :, :, t * P:(t + 1) * P
                    ],
                    in_=cps[:].rearrange("p (c n) -> p c n", c=DCH),
                )

            # ---- x pipeline ----
            scps = []
            egrp = []
            for g in range(ngrp):
                scp_g = sc_psum.tile([P, GRP * K], f32, tag=f"scp{g}")
                scps.append(scp_g)
                e_g = esb_pool.tile([P, GRP * K], f32, tag=f"esb{g}", bufs=1)
                egrp.append(e_g)

            mv = persist.tile([P, 2 * nch], f32)   # (mean, var) for DVE-ss subs
            mv3 = mv[:].rearrange("p (i two) -> p i two", two=2)
            ssq3 = ssq[:].rearrange("p (i one) -> p i one", one=1)

            def quantum(subs):
                # 1) sum of squares per sub-chunk: first half on ACT
                # (Square+accum), second half on DVE (bn_stats).
                h = len(subs) // 2
                for idx, i in enumerate(subs):
                    xfi = xf[:, i * D:(i + 1) * D]
                    if idx < h:
                        ja = junk_pool.tile([P, D], f32, tag="junkA")
                        nc.scalar.activation(
                            out=ja[:], in_=xfi, func=AF.Square,
                            accum_out=ssq[:, i:i + 1],
                        )
                    else:
                        bns = junk_pool.tile([P, 6], f32, tag="bns")
                        nc.vector.bn_stats(out=bns[:], in_=xfi)
                        nc.vector.bn_aggr(out=mv[:, 2 * i:2 * i + 2], in_=bns[:])
                # ss = D*(var + mean^2) for the DVE half (contiguous subs)
                a, b = subs[h], subs[-1] + 1
                nc.vector.tensor_mul(
                    out=ssq3[:, a:b], in0=mv3[:, a:b, 0:1], in1=mv3[:, a:b, 0:1]
                )
                nc.vector.tensor_add(
                    out=ssq3[:, a:b], in0=ssq3[:, a:b], in1=mv3[:, a:b, 1:2]
                )
                nc.vector.tensor_scalar_mul(
                    out=ssq[:, a:b], in0=ssq[:, a:b], scalar1=float(D)
                )
                # 2) batched rsqrt for the quantum: rnorm = exp(-0.5*ln(ss+eps))
                i0, i1 = subs[0], subs[-1] + 1
                nc.vector.tensor_scalar_add(
                    out=ssq[:, i0:i1], in0=ssq[:, i0:i1], scalar1=1e-12
                )
                nc.scalar.activation(
                    out=rnorm[:, i0:i1], in_=ssq[:, i0:i1], func=AF.Ln
                )
                nc.scalar.activation(
                    out=rnorm[:, i0:i1], in_=rnorm[:, i0:i1], func=AF.Exp,
                    scale=-0.5,
                )
                # 3) transpose, scaled pack copy, score matmuls
                for idx, i in enumerate(subs):
                    tps = tp_psum.tile([P, D], f32, tag="tp")
                    for c in range(DCH):
                        nc.tensor.transpose(
                            tps[:, c * P:(c + 1) * P],
                            xf[:, i * D + c * P: i * D + (c + 1) * P],
                            id_f32[:],
                        )
                    xt_dst = xT[:].rearrange("p (c n) -> p c n", c=DCH)[
                        :, :, i * P:(i + 1) * P
                    ]
                    tps_src = tps[:].rearrange("p (c n) -> p c n", c=DCH)
                    if idx % 2 == 1:
                        nc.scalar.activation(
                            out=xt_dst, in_=tps_src, func=AF.Copy,
                        )
                    else:
                        nc.vector.tensor_copy(out=xt_dst, in_=tps_src)
                    j = i // 2          # class/segment of this sub-chunk
                    g = i // GRP
                    sc = scps[g][:, (i - g * GRP) * K:(i - g * GRP + 1) * K]
                    for c in range(DCH):
                        nc.tensor.matmul(
                            sc,
                            xT[:, c * slots + i * P: c * slots + (i + 1) * P],
                            cnT[:, c * ckp + j * K: c * ckp + (j + 1) * K],
                            start=(c == 0),
                            stop=(c == DCH - 1),
                        )
                # 4) per-sub softmax: e = exp(s_raw * rnorm_b) with Z via the
                # ACT accumulator; num_raw = sum_k e_k * s_raw_k batched per
                # group on DVE. rnorm folds in at the tail.
                for i in subs:
                    g = i // GRP
                    sc = scps[g][:, (i - g * GRP) * K:(i - g * GRP + 1) * K]
                    e = egrp[g]
                    nc.scalar.activation(
                        out=e[:, (i - g * GRP) * K:(i - g * GRP + 1) * K],
                        in_=sc, func=AF.Exp,
                        scale=rnorm[:, i:i + 1],
                        accum_out=zsum[:, i:i + 1],
                    )
                    if i == min((g + 1) * GRP, nch) - 1:
                        c0, c1 = g * GRP, min((g + 1) * GRP, nch)
                        gw = c1 - c0
                        jk = junk_pool.tile([P, GRP * K], f32, tag="jk")
                        nc.vector.tensor_mul(
                            out=jk[:, :gw * K], in0=e[:, :gw * K],
                            in1=scps[g][:, :gw * K],
                        )
                        jk3 = jk[:].rearrange("p (i k) -> p i k", k=K)
                        nc.vector.tensor_reduce(
                            out=nums[:, c0:c1], in_=jk3[:, :gw],
                            axis=mybir.AxisListType.X, op=ALU.add,
                        )

            for q in range(npair):
                quantum([4 * q, 4 * q + 1, 4 * q + 2, 4 * q + 3])
            if odd:
                quantum([4 * npair, 4 * npair + 1])

            # ---- tail: t = num_raw * rnorm / Z, partial = sum over slots ----
            nc.vector.reciprocal(out=zsum[:], in_=zsum[:])
            nc.vector.tensor_mul(out=nums[:], in0=nums[:], in1=rnorm[:])
            nc.vector.tensor_mul(out=nums[:], in0=nums[:], in1=zsum[:])
            red = persist.tile([P, 1], f32)
            nc.vector.tensor_reduce(
                out=red[:], in_=nums[:], axis=mybir.AxisListType.X, op=ALU.add,
            )
            fin = fin_psum.tile([1, 1], f32)
            nc.tensor.matmul(fin[:], red[:], ones[:], start=True, stop=True)
            osb = const_pool.tile([1, 1], f32)
            nc.scalar.copy(out=osb[:], in_=fin[:])
            nc.sync.dma_start(out=out[:], in_=osb[:])

    if split_waits:
        _split_excess_waits(nc)
    return nc


def _pack_segments(labels: np.ndarray):
    """Group sample indices by label into segments of <= SEGW, one class per
    segment; pad total segment count to a multiple of NCORES."""
    order = np.argsort(labels, kind="stable")
    sorted_lab = labels[order]
    cut = np.flatnonzero(np.diff(sorted_lab)) + 1
    starts = np.concatenate(([0], cut))
    ends = np.concatenate((cut, [len(labels)]))
    segs = []  # (class, sample_index_array)
    for s, e in zip(starts, ends):
        cls = int(sorted_lab[s])
        for o in range(s, e, SEGW):
            segs.append((cls, order[o:min(o + SEGW, e)]))
    while len(segs) % NCORES != 0:
        segs.append((0, np.empty(0, dtype=np.int64)))
    return segs


def kernel(x: np.ndarray, labels: np.ndarray, centers: np.ndarray) -> np.ndarray:
    x = np.ascontiguousarray(x, dtype=np.float32)
    labels = np.asarray(labels)
    centers = np.ascontiguousarray(centers, dtype=np.float32)
    nb, d = x.shape
    ncls, k, _ = centers.shape
    assert (nb, d, k) == (B, D, K)

    segs = _pack_segments(labels)
    nseg_total = len(segs)
    nseg = nseg_total // NCORES
    slots = nseg * SEGW
    ck = nseg * K
    ckp = ((ck + P - 1) // P) * P

    in_maps = []
    for core in range(NCORES):
        xg = np.zeros((slots, d), dtype=np.float32)
        cent = np.zeros((ckp, d), dtype=np.float32)
        for jj in range(nseg):
            cls, idx = segs[core * nseg + jj]
            if len(idx):
                xg[jj * SEGW: jj * SEGW + len(idx)] = x[idx]
            cent[jj * K:(jj + 1) * K] = centers[cls]
        in_maps.append({"xg": xg, "cent": cent})

    nc = build_bass(nseg)
    res = run_bass_kernel_spmd(nc, in_maps, core_ids=list(range(NCORES)))
    total = sum(float(r["partial"][0, 0]) for r in res.results)
    return np.float32(1.0 - total / nb)
